# revision 1
# baseline (speedup 1.0000x reference)
"""CvT attention block (depthwise conv QKV + MHA) on 8 Trainium2 NeuronCores,
data-parallel over batch. Instruction-count-minimized variant:

  A) depthwise 3x3 convs on DVE: one fused per-partition-scalar FMA
     (scalar_tensor_tensor) per tap with edge-restricted access patterns,
     f32 accumulation scratch, single convert to bf16.
  B) projections in bf16 (moving dim up to 1024): Q^T/K^T in [co, l] layout,
     V-hat in [t, co] layout with a ones column per head (softmax denominators
     fall out of the AV matmul for free).
  C) per (l-chunk of 1024, head): S^T = K_h Q_h^T via PE (two score tiles
     packed in one 4-bank PSUM tile), one Exp over the pair on ACT, AV
     accumulation; then reciprocal + indicator-matmul broadcast for the
     softmax normalization, and the output projection in [co, l] layout with
     a transposing DMA store.
"""

import contextlib
import numpy as np
import ml_dtypes
from concourse import mybir
import concourse.bacc as bacc
import concourse.tile as tile
from concourse.bass_utils import run_bass_kernel_spmd

F32 = mybir.dt.float32
F32R = mybir.dt.float32r
BF16 = mybir.dt.bfloat16
AFT = mybir.ActivationFunctionType
ALU = mybir.AluOpType

C = 384
T = 3136            # 56*56
TKV = 784           # 28*28
NH = 6
SCALE = C ** (-0.5)
EPS = 1e-5
XP = 56 + T + 56

LC = [(i * 512, min(512, T - i * 512)) for i in range(7)]
T_TILES = [(i * 128, min(128, TKV - i * 128)) for i in range(7)]

_CACHE = {}


def _conv(nc, xt, ys, ybf, wb, cv, ch, stride):
    """Depthwise 3x3 conv for one 128-channel chunk on DVE.
    xt: [128, XP] f32 padded input (row r col c of the image lives at flat
    56 + r*56 + c, i.e. x3[1+r, c]).
    ys: f32 scratch [128, out_pix]; ybf: bf16 destination.
    wb: [128, 30] tile; tap t of conv cv at col 9*cv+t, bias at col 27+cv.
    """
    w = lambda t: wb[:, 9 * cv + t:9 * cv + t + 1]
    bias = wb[:, 27 + cv:28 + cv]
    x3 = xt[:, 0:3248].rearrange("p (r c) -> p r c", c=56)  # rows -1..56
    if stride == 1:
        ys3 = ys[:].rearrange("p (r c) -> p r c", c=56)
        # seed with center tap + bias
        nc.vector.tensor_scalar(ys[:], xt[:, 56:56 + T], w(4), bias,
                                op0=ALU.mult, op1=ALU.add)
        for t in (0, 2, 3, 5, 6, 8, 1, 7):
            di, dj = t // 3 - 1, t % 3 - 1
            if dj == 0:
                o = ys[:]
                i = xt[:, 56 + 56 * di:56 + 56 * di + T]
            elif dj < 0:
                o = ys3[:, :, 1:56]
                i = x3[:, 1 + di:57 + di, 0:55]
            else:
                o = ys3[:, :, 0:55]
                i = x3[:, 1 + di:57 + di, 1:56]
            if t == 7:
                o = ybf[:]
                i = xt[:, 56 + 56:56 + 56 + T]
                nc.vector.scalar_tensor_tensor(o, i, w(t), ys[:],
                                               op0=ALU.mult, op1=ALU.add)
            else:
                nc.vector.scalar_tensor_tensor(o, i, w(t), o, op0=ALU.mult,
                                               op1=ALU.add)
    else:
        ysv = ys[:, 0:TKV]
        ys3 = ysv.rearrange("p (r c) -> p r c", c=28)
        nc.vector.tensor_scalar(ysv, x3[:, 1:57:2, 0:56:2], w(4), bias,
                                op0=ALU.mult, op1=ALU.add)
        for t in (0, 1, 2, 3, 5, 6, 8, 7):
            di, dj = t // 3 - 1, t % 3 - 1
            if dj == 0:
                o = ysv
                i = x3[:, 1 + di:57 + di:2, 0:56:2]
            elif dj < 0:
                o = ys3[:, :, 1:28]
                i = x3[:, 1 + di:57 + di:2, 1:54:2]
            else:
                o = ysv
                i = x3[:, 1 + di:57 + di:2, 1:56:2]
            if t == 7:
                nc.vector.scalar_tensor_tensor(ybf[:], i, w(t), ysv,
                                               op0=ALU.mult, op1=ALU.add)
            else:
                nc.vector.scalar_tensor_tensor(o, i, w(t), o, op0=ALU.mult,
                                               op1=ALU.add)


def _emit(nc, tc, ctx, d, reps):
    pers = ctx.enter_context(tc.tile_pool(name="pers", bufs=1))

    wq = [pers.tile([128, C], F32R, tag=f"wq{i}", name=f"wq{i}") for i in range(3)]
    wk = [pers.tile([128, C], F32R, tag=f"wk{i}", name=f"wk{i}") for i in range(3)]
    wvp = [pers.tile([128, NH * 65], F32R, tag=f"wvp{i}", name=f"wvp{i}")
           for i in range(3)]
    wpj = [pers.tile([128, C], F32R, tag=f"wpj{i}", name=f"wpj{i}")
           for i in range(3)]
    ind6 = [pers.tile([6, 128], F32R, tag=f"ind6{i}", name=f"ind6{i}")
            for i in range(3)]
    wb = [pers.tile([128, 30], F32, tag=f"wb{i}", name=f"wb{i}")
          for i in range(3)]
    bpjW = pers.tile([128, 2048], F32, tag="bpjW", name="bpjW")
    QT = [pers.tile([128, T], F32R, tag=f"QT{i}", name=f"QT{i}") for i in range(3)]
    KT = [pers.tile([128, TKV], F32R, tag=f"KT{i}", name=f"KT{i}")
          for i in range(3)]
    Vh = [pers.tile([128, NH * 65], F32R, tag=f"Vh{i}", name=f"Vh{i}")
          for i in range(7)]

    for i in range(3):
        nc.sync.dma_start(wq[i][:], d["wq"][i * 128:(i + 1) * 128, :])
        nc.sync.dma_start(wk[i][:], d["wk"][i * 128:(i + 1) * 128, :])
        nc.sync.dma_start(wvp[i][:], d["wvp"][i * 128:(i + 1) * 128, :])
        nc.sync.dma_start(wpj[i][:], d["wpj"][i * 128:(i + 1) * 128, :])
        nc.sync.dma_start(ind6[i][:], d["ind6"][i])
        nc.sync.dma_start(wb[i][:], d["wb"][i])
    nc.sync.dma_start(bpjW[:], d["bpjW"])

    for rep in range(reps):
        sfx = f"r{rep}"
        with contextlib.ExitStack() as phAB:
            ypool = phAB.enter_context(tc.tile_pool(name="y" + sfx, bufs=1))
            yq = [ypool.tile([128, T], F32R, tag=f"yq{i}", name=f"yq{i}")
                  for i in range(3)]
            yk = [ypool.tile([128, TKV], F32R, tag=f"yk{i}", name=f"yk{i}")
                  for i in range(3)]
            yv = [ypool.tile([128, TKV], F32R, tag=f"yv{i}", name=f"yv{i}")
                  for i in range(3)]

            # ---- Phase A: depthwise convs on DVE ----
            with contextlib.ExitStack() as phA:
                xpool = phA.enter_context(tc.tile_pool(name="x" + sfx, bufs=2))
                spool = xpool
                for ch in range(3):
                    xt = xpool.tile([128, XP], F32, tag="x", name="x")
                    nc.sync.dma_start(xt[:], d["xp"][ch * 128:(ch + 1) * 128, :])
                    ys = spool.tile([128, T], F32, tag="ys", name="ys")
                    _conv(nc, xt, ys, yq[ch], wb[ch], 0, ch, 1)
                    ys = spool.tile([128, T], F32, tag="ys", name="ys")
                    _conv(nc, xt, ys, yk[ch], wb[ch], 1, ch, 2)
                    ys = spool.tile([128, T], F32, tag="ys", name="ys")
                    _conv(nc, xt, ys, yv[ch], wb[ch], 2, ch, 2)

            # ---- Phase B: projections (bf16, 512-wide windows) ----
            with contextlib.ExitStack() as phB:
                psB = phB.enter_context(
                    tc.tile_pool(name="psB" + sfx, bufs=2, space="PSUM"))
                for co in range(3):
                    for g, grp in enumerate((LC[0:4], LC[4:7])):
                        p = psB.tile([128, 2048], F32, tag="psB", name="psB")
                        for k, (lo, ls) in enumerate(grp):
                            for ch in range(3):
                                nc.tensor.matmul(
                                    p[0:128, k * 512:k * 512 + ls],
                                    wq[ch][:, co * 128:(co + 1) * 128],
                                    yq[ch][:, lo:lo + ls],
                                    start=(ch == 0), stop=(ch == 2))
                        base = grp[0][0]
                        wid = grp[-1][0] + grp[-1][1] - base
                        nc.vector.tensor_copy(QT[co][:, base:base + wid],
                                              p[:, 0:wid])
                    p = psB.tile([128, 2048], F32, tag="psB", name="psB")
                    for k, (to, ts) in enumerate(((0, 512), (512, 272))):
                        for ch in range(3):
                            nc.tensor.matmul(
                                p[:, k * 512:k * 512 + ts],
                                wk[ch][:, co * 128:(co + 1) * 128],
                                yk[ch][:, to:to + ts],
                                start=(ch == 0), stop=(ch == 2))
                    nc.vector.tensor_copy(KT[co][:], p[:, 0:TKV])
                for gi in range(2):
                    tt = T_TILES[4 * gi:4 * gi + 4]
                    p = psB.tile([128, 2048], F32, tag="psB", name="psB")
                    for k, (to, ts) in enumerate(tt):
                        for ch in range(3):
                            nc.tensor.matmul(
                                p[0:ts, k * 512:k * 512 + NH * 65],
                                yv[ch][:, to:to + ts], wvp[ch][:],
                                start=(ch == 0), stop=(ch == 2))
                    for k, (to, ts) in enumerate(tt):
                        ti = 4 * gi + k
                        nc.vector.tensor_copy(
                            Vh[ti][0:ts, :], p[0:ts, k * 512:k * 512 + NH * 65])
                        nc.vector.memset(Vh[ti][0:ts, 64:NH * 65:65].bitcast(F32), 1.0)

            if "dbg_yq" in d:
                for i in range(3):
                    nc.sync.dma_start(d["dbg_yq"][i], yq[i][:])
                    nc.sync.dma_start(d["dbg_yk"][i], yk[i][:])
                    nc.sync.dma_start(d["dbg_qt"][i], QT[i][:])
                    nc.sync.dma_start(d["dbg_kt"][i], KT[i][:])
                for i in range(7):
                    nc.sync.dma_start(d["dbg_vh"][i], Vh[i][:])

        # ---- Phase C: attention + normalize + output projection ----
        with contextlib.ExitStack() as phC:
            cw = phC.enter_context(tc.tile_pool(name="cw" + sfx, bufs=2))
            etp = otp = rcp = outp = cw
            psS = phC.enter_context(
                tc.tile_pool(name="psS" + sfx, bufs=1, space="PSUM"))
            psO = phC.enter_context(
                tc.tile_pool(name="psO" + sfx, bufs=2, space="PSUM"))

            for lo, ls in LC:
                OTb = otp.tile([128, 1536], F32R, tag="otb", name="otb")
                rcf = rcp.tile([1, NH * 512], F32, tag="rcf", name="rcf")
                rc6 = rcp.tile([6, 512], F32, tag="rc6", name="rc6")
                rc6r = rcp.tile([6, 512], F32R, tag="rc6r", name="rc6r")
                for h in range(NH):
                    c2, po = h // 2, 64 * (h % 2)
                    ets = []
                    for gi, tt in enumerate((T_TILES[0:4], T_TILES[4:7])):
                        p = psS.tile([128, 2048], F32, tag="psS", name="psS")
                        for k, (to, ts) in enumerate(tt):
                            nc.tensor.matmul(p[0:ts, k * 512:k * 512 + ls],
                                             KT[c2][po:po + 64, to:to + ts],
                                             QT[c2][po:po + 64, lo:lo + ls],
                                             start=True, stop=True)
                        et = etp.tile([128, 2048], F32R, tag="et", name="et", bufs=4)
                        wid = (len(tt) - 1) * 512 + ls
                        nc.scalar.activation(et[:, 0:wid], p[:, 0:wid],
                                             AFT.Exp, scale=float(SCALE))
                        ets.append(et)
                    po2 = psO.tile([65, 512], F32, tag="psO", name="psO")
                    for ti, (to, ts) in enumerate(T_TILES):
                        nc.tensor.matmul(
                            po2[:, :ls], Vh[ti][0:ts, h * 65:(h + 1) * 65],
                            ets[ti // 4][0:ts, (ti % 4) * 512:(ti % 4) * 512 + ls],
                            start=(ti == 0), stop=(ti == 6))
                    nc.vector.tensor_copy(
                        OTb[po:po + 64, c2 * 512:c2 * 512 + ls], po2[0:64, :ls])
                    nc.vector.tensor_copy(rcf[0:1, h * 512:h * 512 + ls],
                                          po2[64:65, :ls])
                # gather sums -> 6 partitions, recip, indicator bcast, scale
                rin = rcf[0:1, :].rearrange("p (g l) -> p g l", l=512)
                nc.sync.dma_start(rc6[0:6, 0:ls], rin[:, :, 0:ls])
                with nc.allow_low_precision(reason="f32r recip"):
                    nc.vector.reciprocal(rc6r[0:6, 0:ls], rc6[0:6, 0:ls])
                if "dbg_rc6" in d and lo == 0:
                    nc.sync.dma_start(d["dbg_rc6"], rc6[:])
                rbp = psS.tile([128, 2048], F32, tag="psS", name="psS")
                for ch in range(3):
                    nc.tensor.matmul(rbp[:, ch * 512:ch * 512 + ls],
                                     ind6[ch][:], rc6r[0:6, 0:ls],
                                     start=True, stop=True)
                o3 = OTb[:].rearrange("p (w l) -> p w l", l=512)[:, :, 0:ls]
                r3 = rbp[:].rearrange("p (w l) -> p w l", l=512)[:, 0:3, 0:ls]
                nc.vector.tensor_mul(o3, o3, r3)
                if "dbg_otb" in d and lo == 0:
                    nc.sync.dma_start(d["dbg_otb"], OTb[:])
                # output projection: [l, co] layout, 4 l-tile windows
                p = psS.tile([128, 2048], F32, tag="psS", name="psS")
                nlt = (ls + 127) // 128
                for k in range(nlt):
                    lsz = min(128, ls - k * 128)
                    win = p[0:lsz, k * 512:k * 512 + C]
                    for ch in range(3):
                        nc.tensor.matmul(
                            win, OTb[:, ch * 512 + k * 128:
                                     ch * 512 + k * 128 + lsz],
                            wpj[ch][:], start=(ch == 0), stop=(ch == 2))
                osb = outp.tile([128, 2048], F32, tag="o", name="o")
                ov = osb[:].rearrange("p (w c) -> p w c", c=512)[:, 0:nlt, 0:C]
                pv = p[:].rearrange("p (w c) -> p w c", c=512)[:, 0:nlt, 0:C]
                bv = bpjW[:].rearrange("p (w c) -> p w c", c=512)[:, 0:nlt, 0:C]
                nc.vector.tensor_add(ov, pv, bv)
                lsz = min(128, ls - (nlt - 1) * 128)
                if lsz < 128:
                    ov = ov[0:lsz]
                dst = d["out"][lo:lo + ls, :].rearrange(
                    "(w p) c -> p w c", p=min(128, ls))
                nc.sync.dma_start(dst, ov)


def _build(reps=1):
    if reps in _CACHE:
        return _CACHE[reps]
    nc = bacc.Bacc("TRN2", target_bir_lowering=False, debug=False)
    d = {
        "xp": nc.dram_tensor("xp", [C, XP], F32, kind="ExternalInput").ap(),
        "wb": nc.dram_tensor("wb", [3, 128, 30], F32, kind="ExternalInput").ap(),
        "wq": nc.dram_tensor("wq", [C, C], F32R, kind="ExternalInput").ap(),
        "wk": nc.dram_tensor("wk", [C, C], F32R, kind="ExternalInput").ap(),
        "wvp": nc.dram_tensor("wvp", [C, NH * 65], F32R,
                              kind="ExternalInput").ap(),
        "wpj": nc.dram_tensor("wpj", [C, C], F32R, kind="ExternalInput").ap(),
        "ind6": nc.dram_tensor("ind6", [3, 6, 128], F32R,
                               kind="ExternalInput").ap(),
        "bpjW": nc.dram_tensor("bpjW", [128, 2048], F32, kind="ExternalInput").ap(),
        "out": nc.dram_tensor("out", [T, C], F32, kind="ExternalOutput").ap(),
    }
    with tile.TileContext(nc) as tc:
        with contextlib.ExitStack() as ctx:
            _emit(nc, tc, ctx, d, reps)
    nc.compile()
    _CACHE[reps] = nc
    return nc


def _bpjw(bproj):
    w = np.zeros((128, 2048), np.float32)
    for k in range(4):
        w[:, k * 512:k * 512 + C] = bproj[None, :]
    return w


def _host_prep(x, conv_q, conv_k, conv_v, bn_q, bn_k, bn_v, Wq, Wk, Wv,
               Wproj, bproj):
    B = x.shape[0]
    x = np.asarray(x, np.float32)
    xp = np.zeros((B, C, XP), np.float32)
    xp[:, :, 56:56 + T] = np.ascontiguousarray(x.transpose(0, 2, 1))

    wb = np.zeros((3, 128, 30), np.float32)
    for cv, (w, bn) in enumerate(((conv_q, bn_q), (conv_k, bn_k),
                                  (conv_v, bn_v))):
        g, b, m, v = [np.asarray(bn[i], np.float64) for i in range(4)]
        a = g / np.sqrt(v + EPS)
        bias = (b - m * a).astype(np.float32)
        wh = (np.asarray(w, np.float64).reshape(C, 9) * a[:, None]).astype(
            np.float32)
        for ch in range(3):
            wb[ch, :, 9 * cv:9 * cv + 9] = wh[ch * 128:(ch + 1) * 128]
            wb[ch, :, 27 + cv] = bias[ch * 128:(ch + 1) * 128]

    wvp = np.zeros((C, NH * 65), np.float32)
    Wv = np.asarray(Wv, np.float32)
    for h in range(NH):
        wvp[:, h * 65:h * 65 + 64] = Wv[:, h * 64:(h + 1) * 64]

    ind6 = np.zeros((3, 6, 128), np.float32)
    for ch in range(3):
        ind6[ch, 2 * ch, 0:64] = 1.0
        ind6[ch, 2 * ch + 1, 64:128] = 1.0

    return {
        "xp": xp,
        "wb": wb,
        "wq": np.asarray(Wq, np.float32),
        "wk": np.asarray(Wk, np.float32),
        "wvp": wvp,
        "wpj": np.asarray(Wproj, np.float32),
        "ind6": ind6,
        "bpjW": _bpjw(np.asarray(bproj, np.float32)),
    }


def kernel(x, h, w, conv_q, conv_k, conv_v, bn_q, bn_k, bn_v, Wq, Wk, Wv,
           Wproj, bproj, _reps=1, _nc=None):
    B = x.shape[0]
    nc = _nc if _nc is not None else _build(_reps)
    hp = _host_prep(x, conv_q, conv_k, conv_v, bn_q, bn_k, bn_v, Wq, Wk, Wv,
                    Wproj, bproj)
    shared = {k: v for k, v in hp.items() if k != "xp"}
    in_maps = [dict(shared, xp=hp["xp"][b]) for b in range(B)]
    res = run_bass_kernel_spmd(nc, in_maps, core_ids=list(range(B)))
    out = np.stack([res.results[b]["out"] for b in range(B)], axis=0)
    return out.astype(np.float32)



# revision 6
# speedup vs baseline: 148.7476x; 148.7476x over previous
"""CvT attention block (depthwise conv QKV + MHA) on 8 Trainium2 NeuronCores,
data-parallel over batch.

v3 (bf16 matmul path, PE-resident schedule):
  A) input staged as a 58x58 zero-padded bf16 image (row stride 58, data at
     rows/cols 1..56) so every conv tap is a full-range shifted view with no
     edge cases.  q-conv (stride 1) runs on the PE as 9 accumulating
     diagonal-weight matmuls per 8-row chunk (keeps the PE HAM-warm from the
     start); k/v convs (stride 2) run on DVE as f32 FMA chains.
  B) projections in bf16: Q^T/K^T in [co, l] layout, V-hat in [t, co]
     layout with a ones column per head (softmax denominators fall out of
     the AV matmul for free).
  C) per (l-chunk of 512, head): S^T = K_h Q_h^T in bf16 into two
     independent PSUM tiles (4-bank + 3-bank) so ACT exp of one tile
     overlaps PE matmuls of the other; exp writes bf16; AV accumulates into
     a shared 1-bank PSUM tile; softmax normalization via fast-reciprocal +
     2x128 indicator matmuls (3 pieces through the shared bank); output
     projection windows through the same shared bank, fused bias-add on the
     PSUM->SBUF copy, transposing DMA store.  PSUM: 4+3+1 = 8 banks, so the
     next l-chunk's score matmuls never wait on the normalization tail.
"""

import contextlib
import numpy as np
import ml_dtypes
from concourse import mybir
import concourse.bacc as bacc
import concourse.tile as tile
from concourse.bass_utils import run_bass_kernel_spmd

F32 = mybir.dt.float32
BF16 = mybir.dt.bfloat16
AFT = mybir.ActivationFunctionType
ALU = mybir.AluOpType

C = 384
T = 3136            # 56*56
TKV = 784           # 28*28
NH = 6
SCALE = C ** (-0.5)
EPS = 1e-5
XB = 3368           # 2 + 58*58 + 8 slack; data (r,c) at 2 + (1+r)*58 + 1+c

LC = [(i * 512, min(512, T - i * 512)) for i in range(7)]
T_TILES = [(i * 128, min(128, TKV - i * 128)) for i in range(7)]
TAPS = [(t // 3 - 1, t % 3 - 1) for t in range(9)]

_CACHE = {}


def _conv2p(nc, eng, xb, ys, ybf, wb, cv):
    """Stride-2 depthwise 3x3 conv (28x28 out) from the 58-padded bf16 image.
    All taps are full-range shifted strided views (padding absorbs edges)."""
    w = lambda t: wb[:, 9 * cv + t:9 * cv + t + 1]
    bias = wb[:, 27 + cv:28 + cv]
    x3 = xb[:, 2:2 + 3364].rearrange("p (r c) -> p r c", c=58)
    ysv = ys[:, 0:TKV]
    tap = lambda di, dj: x3[:, 1 + di:57 + di:2, 1 + dj:57 + dj:2]
    eng.tensor_scalar(ysv, tap(0, 0), w(4), bias, op0=ALU.mult, op1=ALU.add)
    for t in (0, 1, 2, 3, 5, 6, 8, 7):
        di, dj = TAPS[t]
        if t == 7:
            eng.scalar_tensor_tensor(ybf[:], tap(di, dj), w(t), ysv,
                                     op0=ALU.mult, op1=ALU.add)
        else:
            eng.scalar_tensor_tensor(ysv, tap(di, dj), w(t), ysv,
                                     op0=ALU.mult, op1=ALU.add)


def _emit(nc, tc, ctx, d, reps):
    pers = ctx.enter_context(tc.tile_pool(name="pers", bufs=1))

    wq = [pers.tile([128, C], BF16, tag=f"wq{i}", name=f"wq{i}") for i in range(3)]
    wk = [pers.tile([128, C], BF16, tag=f"wk{i}", name=f"wk{i}") for i in range(3)]
    wvp = [pers.tile([128, NH * 65], BF16, tag=f"wvp{i}", name=f"wvp{i}")
           for i in range(3)]
    wpj = [pers.tile([128, C], BF16, tag=f"wpj{i}", name=f"wpj{i}")
           for i in range(3)]
    wdq = [pers.tile([128, 9 * 128], BF16, tag=f"wdq{i}", name=f"wdq{i}")
           for i in range(3)]
    ind2 = pers.tile([2, 128], BF16, tag="ind2", name="ind2")
    wb = [pers.tile([128, 30], F32, tag=f"wb{i}", name=f"wb{i}")
          for i in range(3)]
    bpjW = pers.tile([128, 2048], F32, tag="bpjW", name="bpjW")
    QT = [pers.tile([128, T], BF16, tag=f"QT{i}", name=f"QT{i}") for i in range(3)]
    KT = [pers.tile([128, TKV], BF16, tag=f"KT{i}", name=f"KT{i}")
          for i in range(3)]
    Vh = [pers.tile([128, NH * 65], BF16, tag=f"Vh{i}", name=f"Vh{i}")
          for i in range(7)]

    for i in range(3):
        nc.sync.dma_start(wq[i][:], d["wq"][i * 128:(i + 1) * 128, :])
        nc.sync.dma_start(wk[i][:], d["wk"][i * 128:(i + 1) * 128, :])
        nc.sync.dma_start(wvp[i][:], d["wvp"][i * 128:(i + 1) * 128, :])
        nc.sync.dma_start(wpj[i][:], d["wpj"][i * 128:(i + 1) * 128, :])
        nc.sync.dma_start(wdq[i][:], d["wdq"][i])
        nc.sync.dma_start(wb[i][:], d["wb"][i])
    nc.sync.dma_start(ind2[:], d["ind2"])
    nc.sync.dma_start(bpjW[:], d["bpjW"])

    for rep in range(reps):
        sfx = f"r{rep}"
        with contextlib.ExitStack() as phAB:
            ypool = phAB.enter_context(tc.tile_pool(name="y" + sfx, bufs=1))
            yq = [ypool.tile([128, T], BF16, tag=f"yq{i}", name=f"yq{i}")
                  for i in range(3)]
            yk = [ypool.tile([128, TKV], BF16, tag=f"yk{i}", name=f"yk{i}")
                  for i in range(3)]
            yv = [ypool.tile([128, TKV], BF16, tag=f"yv{i}", name=f"yv{i}")
                  for i in range(3)]
            psAB = phAB.enter_context(
                tc.tile_pool(name="psAB" + sfx, bufs=1, space="PSUM"))
            xpool = phAB.enter_context(tc.tile_pool(name="x" + sfx, bufs=2))

            # ---- Phase A: q-conv on PE (diag matmuls), k/v convs on DVE ----
            for ch in range(3):
                xb = xpool.tile([128, XB], BF16, tag="x", name="x")
                nc.sync.dma_start(xb[:], d["xb"][ch * 128:(ch + 1) * 128, :])
                biasq = wb[ch][:, 27:28]
                for k in range(7):
                    pcv = psAB.tile([128, 512], F32, tag="pcv", name="pcv",
                                    bufs=4)
                    base = 2 + (1 + 8 * k) * 58
                    for t, (di, dj) in enumerate(TAPS):
                        off = base + 58 * di + dj
                        nc.tensor.matmul(pcv[:, 0:464],
                                         wdq[ch][:, t * 128:(t + 1) * 128],
                                         xb[:, off:off + 464],
                                         start=(t == 0), stop=(t == 8))
                    src = pcv[:, 0:464].rearrange(
                        "p (r c) -> p r c", c=58)[:, :, 1:57]
                    dst = yq[ch][:, 448 * k:448 * (k + 1)].rearrange(
                        "p (r c) -> p r c", c=56)
                    nc.vector.tensor_scalar_add(dst, src, biasq)
                ysk = xpool.tile([128, TKV], F32, tag="ysk", name="ysk")
                _conv2p(nc, nc.vector, xb, ysk, yk[ch], wb[ch], 1)
                ysv = xpool.tile([128, TKV], F32, tag="ysv", name="ysv")
                _conv2p(nc, nc.vector, xb, ysv, yv[ch], wb[ch], 2)

            # ---- Phase B: projections (bf16, 512-wide windows) ----
            for co in range(3):
                for g, grp in enumerate((LC[0:4], LC[4:7])):
                    p = psAB.tile([128, 2048], F32, tag="psB", name="psB")
                    for k, (lo, ls) in enumerate(grp):
                        for ch in range(3):
                            nc.tensor.matmul(
                                p[0:128, k * 512:k * 512 + ls],
                                wq[ch][:, co * 128:(co + 1) * 128],
                                yq[ch][:, lo:lo + ls],
                                start=(ch == 0), stop=(ch == 2))
                    base = grp[0][0]
                    wid = grp[-1][0] + grp[-1][1] - base
                    nc.vector.tensor_copy(QT[co][:, base:base + wid],
                                          p[:, 0:wid])
                p = psAB.tile([128, 2048], F32, tag="psB", name="psB")
                for k, (to, ts) in enumerate(((0, 512), (512, 272))):
                    for ch in range(3):
                        nc.tensor.matmul(
                            p[:, k * 512:k * 512 + ts],
                            wk[ch][:, co * 128:(co + 1) * 128],
                            yk[ch][:, to:to + ts],
                            start=(ch == 0), stop=(ch == 2))
                nc.vector.tensor_copy(KT[co][:], p[:, 0:TKV])
            for gi in range(2):
                tt = T_TILES[4 * gi:4 * gi + 4]
                p = psAB.tile([128, 2048], F32, tag="psB", name="psB")
                for k, (to, ts) in enumerate(tt):
                    for ch in range(3):
                        nc.tensor.matmul(
                            p[0:ts, k * 512:k * 512 + NH * 65],
                            yv[ch][:, to:to + ts], wvp[ch][:],
                            start=(ch == 0), stop=(ch == 2))
                for k, (to, ts) in enumerate(tt):
                    ti = 4 * gi + k
                    nc.vector.tensor_copy(
                        Vh[ti][0:ts, :], p[0:ts, k * 512:k * 512 + NH * 65])
                    nc.vector.memset(Vh[ti][0:ts, 64:NH * 65:65], 1.0)

        # ---- Phase C: attention + normalize + output projection ----
        with contextlib.ExitStack() as phC:
            cw = phC.enter_context(tc.tile_pool(name="cw" + sfx, bufs=2))
            psS = phC.enter_context(
                tc.tile_pool(name="psS" + sfx, bufs=1, space="PSUM"))

            for lo, ls in LC:
                OTb = cw.tile([128, 1536], BF16, tag="otb", name="otb")
                rcf = cw.tile([1, NH * 512], F32, tag="rcf", name="rcf")
                rc2 = cw.tile([2, 1536], F32, tag="rc2", name="rc2")
                rcA = cw.tile([2, 1536], F32, tag="rcA", name="rcA")
                rc2r = cw.tile([2, 1536], BF16, tag="rc2r", name="rc2r")
                for h in range(NH):
                    c2, po = h // 2, 64 * (h % 2)
                    ets = []
                    for gi, tt in enumerate((T_TILES[0:4], T_TILES[4:7])):
                        wid = (len(tt) - 1) * 512 + ls
                        p = psS.tile([128, 2048 if gi == 0 else 1536], F32,
                                     tag=("psA" if gi == 0 else "psSB"),
                                     name="psS")
                        for k, (to, ts) in enumerate(tt):
                            nc.tensor.matmul(p[0:ts, k * 512:k * 512 + ls],
                                             KT[c2][po:po + 64, to:to + ts],
                                             QT[c2][po:po + 64, lo:lo + ls],
                                             start=True, stop=True)
                        et = cw.tile([128, 2048], BF16, tag="et", name="et",
                                     bufs=4)
                        nc.scalar.activation(et[:, 0:wid], p[:, 0:wid],
                                             AFT.Exp, scale=float(SCALE))
                        ets.append(et)
                    po2 = psS.tile([128, 512], F32, tag="ps1", name="ps1")
                    for ti, (to, ts) in enumerate(T_TILES):
                        nc.tensor.matmul(
                            po2[0:65, 0:ls], Vh[ti][0:ts, h * 65:(h + 1) * 65],
                            ets[ti // 4][0:ts, (ti % 4) * 512:(ti % 4) * 512 + ls],
                            start=(ti == 0), stop=(ti == 6))
                    nc.vector.tensor_copy(
                        OTb[po:po + 64, c2 * 512:c2 * 512 + ls], po2[0:64, 0:ls])
                    nc.vector.tensor_copy(rcf[0:1, h * 512:h * 512 + ls],
                                          po2[64:65, 0:ls])
                # denominators -> [2, 3*512] (even heads row 0, odd row 1),
                # fast reciprocal, indicator bcast + scale per co-block
                rcf3 = rcf[0:1, :].rearrange("p (g l) -> p g l", l=1024)
                rc2v = rc2[:].rearrange("p (g l) -> p g l", l=512)
                for q in range(2):
                    nc.sync.dma_start(rc2v[q:q + 1, :, 0:ls],
                                      rcf3[:, :, 512 * q:512 * q + ls])
                nc.vector.reciprocal_approx_fast(rcA[:], rc2[:])
                with nc.allow_low_precision(reason="bf16 softmax recip"):
                    nc.vector.tensor_copy(rc2r[:], rcA[:])
                for c2 in range(3):
                    rb = psS.tile([128, 512], F32, tag="ps1", name="ps1")
                    nc.tensor.matmul(rb[:, 0:ls], ind2[:],
                                     rc2r[0:2, c2 * 512:c2 * 512 + ls],
                                     start=True, stop=True)
                    nc.vector.tensor_mul(OTb[:, c2 * 512:c2 * 512 + ls],
                                         OTb[:, c2 * 512:c2 * 512 + ls],
                                         rb[:, 0:ls])
                # output projection: [l, co] windows through the shared bank
                osb = cw.tile([128, 2048], F32, tag="o", name="o")
                nlt = (ls + 127) // 128
                for k in range(nlt):
                    lsz = min(128, ls - k * 128)
                    win = psS.tile([128, 512], F32, tag="ps1", name="ps1")
                    for ch in range(3):
                        nc.tensor.matmul(
                            win[0:lsz, 0:C],
                            OTb[:, ch * 512 + k * 128:ch * 512 + k * 128 + lsz],
                            wpj[ch][:], start=(ch == 0), stop=(ch == 2))
                    nc.vector.tensor_add(osb[0:lsz, k * 512:k * 512 + C],
                                         win[0:lsz, 0:C],
                                         bpjW[0:lsz, k * 512:k * 512 + C])
                ov = osb[:].rearrange("p (w c) -> p w c", c=512)[:, 0:nlt, 0:C]
                lsz = min(128, ls - (nlt - 1) * 128)
                if lsz < 128:
                    ov = ov[0:lsz]
                dst = d["out"][lo:lo + ls, :].rearrange(
                    "(w p) c -> p w c", p=min(128, ls))
                nc.sync.dma_start(dst, ov)


def _build(reps=1):
    if reps in _CACHE:
        return _CACHE[reps]
    nc = bacc.Bacc("TRN2", target_bir_lowering=False, debug=False)
    d = {
        "xb": nc.dram_tensor("xb", [C, XB], BF16, kind="ExternalInput").ap(),
        "wb": nc.dram_tensor("wb", [3, 128, 30], F32, kind="ExternalInput").ap(),
        "wdq": nc.dram_tensor("wdq", [3, 128, 9 * 128], BF16,
                              kind="ExternalInput").ap(),
        "wq": nc.dram_tensor("wq", [C, C], BF16, kind="ExternalInput").ap(),
        "wk": nc.dram_tensor("wk", [C, C], BF16, kind="ExternalInput").ap(),
        "wvp": nc.dram_tensor("wvp", [C, NH * 65], BF16,
                              kind="ExternalInput").ap(),
        "wpj": nc.dram_tensor("wpj", [C, C], BF16, kind="ExternalInput").ap(),
        "ind2": nc.dram_tensor("ind2", [2, 128], BF16,
                               kind="ExternalInput").ap(),
        "bpjW": nc.dram_tensor("bpjW", [128, 2048], F32, kind="ExternalInput").ap(),
        "out": nc.dram_tensor("out", [T, C], F32, kind="ExternalOutput").ap(),
    }
    with tile.TileContext(nc) as tc:
        with contextlib.ExitStack() as ctx:
            _emit(nc, tc, ctx, d, reps)
    nc.compile()
    _CACHE[reps] = nc
    return nc


def _bpjw(bproj):
    w = np.zeros((128, 2048), np.float32)
    for k in range(4):
        w[:, k * 512:k * 512 + C] = bproj[None, :]
    return w


def _host_prep(x, conv_q, conv_k, conv_v, bn_q, bn_k, bn_v, Wq, Wk, Wv,
               Wproj, bproj):
    bf = ml_dtypes.bfloat16
    B = x.shape[0]
    x = np.asarray(x, np.float32)
    # 58x58 zero-padded bf16 image: data (r,c) at col 2 + (1+r)*58 + 1+c
    xb = np.zeros((B, C, XB), bf)
    xi = np.ascontiguousarray(x.transpose(0, 2, 1)).reshape(B, C, 56, 56)
    xb3 = xb[:, :, 2:2 + 3364].reshape(B, C, 58, 58)
    xb3[:, :, 1:57, 1:57] = xi.astype(bf)

    wb = np.zeros((3, 128, 30), np.float32)
    whs = []
    for cv, (w, bn) in enumerate(((conv_q, bn_q), (conv_k, bn_k),
                                  (conv_v, bn_v))):
        g, b, m, v = [np.asarray(bn[i], np.float64) for i in range(4)]
        a = g / np.sqrt(v + EPS)
        bias = (b - m * a).astype(np.float32)
        wh = (np.asarray(w, np.float64).reshape(C, 9) * a[:, None]).astype(
            np.float32)
        whs.append(wh)
        for ch in range(3):
            wb[ch, :, 9 * cv:9 * cv + 9] = wh[ch * 128:(ch + 1) * 128]
            wb[ch, :, 27 + cv] = bias[ch * 128:(ch + 1) * 128]

    # diag-packed q-conv weights for the PE: wdq[ch][p, t*128+q] = d_pq*wh[q,t]
    wdq = np.zeros((3, 128, 9 * 128), np.float32)
    idx = np.arange(128)
    for ch in range(3):
        for t in range(9):
            wdq[ch, idx, t * 128 + idx] = whs[0][ch * 128 + idx, t]

    wvp = np.zeros((C, NH * 65), np.float32)
    Wv = np.asarray(Wv, np.float32)
    for h in range(NH):
        wvp[:, h * 65:h * 65 + 64] = Wv[:, h * 64:(h + 1) * 64]

    ind2 = np.zeros((2, 128), np.float32)
    ind2[0, 0:64] = 1.0
    ind2[1, 64:128] = 1.0

    return {
        "xb": xb,
        "wb": wb,
        "wdq": wdq.astype(bf),
        "wq": np.asarray(Wq, np.float32).astype(bf),
        "wk": np.asarray(Wk, np.float32).astype(bf),
        "wvp": wvp.astype(bf),
        "wpj": np.asarray(Wproj, np.float32).astype(bf),
        "ind2": ind2.astype(bf),
        "bpjW": _bpjw(np.asarray(bproj, np.float32)),
    }


def kernel(x, h, w, conv_q, conv_k, conv_v, bn_q, bn_k, bn_v, Wq, Wk, Wv,
           Wproj, bproj, _reps=1, _nc=None):
    B = x.shape[0]
    nc = _nc if _nc is not None else _build(_reps)
    hp = _host_prep(x, conv_q, conv_k, conv_v, bn_q, bn_k, bn_v, Wq, Wk, Wv,
                    Wproj, bproj)
    shared = {k: v for k, v in hp.items() if k != "xb"}
    in_maps = [dict(shared, xb=hp["xb"][b]) for b in range(B)]
    res = run_bass_kernel_spmd(nc, in_maps, core_ids=list(range(B)))
    out = np.stack([res.results[b]["out"] for b in range(B)], axis=0)
    return out.astype(np.float32)


# revision 7
# speedup vs baseline: 161.1009x; 1.0830x over previous
"""CvT attention block (depthwise conv QKV + MHA) on 8 Trainium2 NeuronCores,
data-parallel over batch.

v4 (bf16 matmul path, PE-resident schedule):
  A) input staged as a 58x58 zero-padded bf16 image (row stride 58, data at
     rows/cols 1..56) so every conv tap is a full-range shifted view with no
     edge cases.  ALL depthwise convs run on the PE as 9 accumulating
     diagonal-weight matmuls per output chunk (q: 7x 8-row chunks; k/v:
     2x 14-row strided chunks each); DVE only does the PSUM->SBUF
     extraction with fused BN-bias add and bf16 cast.  Keeps the PE
     HAM-warm from the start and off-loads DVE.
  B) projections in bf16 with double-buffered [128,1024] PSUM tiles:
     Q^T/K^T in [co, l] layout, V-hat in [t, co] layout with a ones column
     per head (softmax denominators fall out of the AV matmul for free).
  C) per (l-chunk of 512, head): S^T = K_h Q_h^T in bf16 into two
     independent PSUM tiles (4-bank + 3-bank) so ACT exp of one tile
     overlaps PE matmuls of the other; exp writes bf16; AV accumulates into
     a shared 1-bank PSUM tile.  The per-chunk tail (softmax normalization
     via fast reciprocal + 2x128 indicator matmuls, output-projection
     windows with fused bias-add, transposing DMA store) is emitted inside
     the NEXT l-chunk's head loop, so the in-order PE queue never
     head-of-line blocks on the tail's DVE/DMA dependencies.
"""

import contextlib
import numpy as np
import ml_dtypes
from concourse import mybir
import concourse.bacc as bacc
import concourse.tile as tile
from concourse.bass_utils import run_bass_kernel_spmd

F32 = mybir.dt.float32
BF16 = mybir.dt.bfloat16
AFT = mybir.ActivationFunctionType
ALU = mybir.AluOpType

C = 384
T = 3136            # 56*56
TKV = 784           # 28*28
NH = 6
SCALE = C ** (-0.5)
EPS = 1e-5
XB = 3368           # 2 + 58*58 + 8 slack; data (r,c) at 2 + (1+r)*58 + 1+c

LC = [(i * 512, min(512, T - i * 512)) for i in range(7)]
T_TILES = [(i * 128, min(128, TKV - i * 128)) for i in range(7)]
TAPS = [(t // 3 - 1, t % 3 - 1) for t in range(9)]

_CACHE = {}


def _emit(nc, tc, ctx, d, reps):
    pers = ctx.enter_context(tc.tile_pool(name="pers", bufs=1))

    wq = [pers.tile([128, C], BF16, tag=f"wq{i}", name=f"wq{i}") for i in range(3)]
    wk = [pers.tile([128, C], BF16, tag=f"wk{i}", name=f"wk{i}") for i in range(3)]
    wvp = [pers.tile([128, NH * 65], BF16, tag=f"wvp{i}", name=f"wvp{i}")
           for i in range(3)]
    wpj = [pers.tile([128, C], BF16, tag=f"wpj{i}", name=f"wpj{i}")
           for i in range(3)]
    wd = [pers.tile([128, 27 * 128], BF16, tag=f"wd{i}", name=f"wd{i}")
          for i in range(3)]
    ind2 = pers.tile([2, 128], BF16, tag="ind2", name="ind2")
    wb = [pers.tile([128, 30], F32, tag=f"wb{i}", name=f"wb{i}")
          for i in range(3)]
    bpjW = pers.tile([128, 2048], F32, tag="bpjW", name="bpjW")
    QT = [pers.tile([128, T], BF16, tag=f"QT{i}", name=f"QT{i}") for i in range(3)]
    KT = [pers.tile([128, TKV], BF16, tag=f"KT{i}", name=f"KT{i}")
          for i in range(3)]
    Vh = [pers.tile([128, NH * 65], BF16, tag=f"Vh{i}", name=f"Vh{i}")
          for i in range(7)]

    for i in range(3):
        nc.sync.dma_start(wq[i][:], d["wq"][i * 128:(i + 1) * 128, :])
        nc.sync.dma_start(wk[i][:], d["wk"][i * 128:(i + 1) * 128, :])
        nc.sync.dma_start(wvp[i][:], d["wvp"][i * 128:(i + 1) * 128, :])
        nc.sync.dma_start(wpj[i][:], d["wpj"][i * 128:(i + 1) * 128, :])
        nc.sync.dma_start(wd[i][:], d["wd"][i])
        nc.sync.dma_start(wb[i][:], d["wb"][i])
    nc.sync.dma_start(ind2[:], d["ind2"])
    nc.sync.dma_start(bpjW[:], d["bpjW"])

    for rep in range(reps):
        sfx = f"r{rep}"
        with contextlib.ExitStack() as phAB:
            ypool = phAB.enter_context(tc.tile_pool(name="y" + sfx, bufs=1))
            yq = [ypool.tile([128, T], BF16, tag=f"yq{i}", name=f"yq{i}")
                  for i in range(3)]
            yk = [ypool.tile([128, TKV], BF16, tag=f"yk{i}", name=f"yk{i}")
                  for i in range(3)]
            yv = [ypool.tile([128, TKV], BF16, tag=f"yv{i}", name=f"yv{i}")
                  for i in range(3)]
            psAB = phAB.enter_context(
                tc.tile_pool(name="psAB" + sfx, bufs=1, space="PSUM"))
            xpool = phAB.enter_context(tc.tile_pool(name="x" + sfx, bufs=2))

            # ---- Phase A: all convs on PE via diagonal-weight matmuls ----
            for ch in range(3):
                xb = xpool.tile([128, XB], BF16, tag="x", name="x")
                nc.sync.dma_start(xb[:], d["xb"][ch * 128:(ch + 1) * 128, :])
                x3 = xb[:, 2:2 + 3364].rearrange("p (r c) -> p r c", c=58)
                # q: stride 1, 7 chunks of 8 rows (464 padded cols each)
                for k in range(7):
                    pcv = psAB.tile([128, 512], F32, tag="pcv", name="pcv",
                                    bufs=4)
                    base = 2 + (1 + 8 * k) * 58
                    for t, (di, dj) in enumerate(TAPS):
                        nc.tensor.matmul(pcv[:, 0:464],
                                         wd[ch][:, t * 128:(t + 1) * 128],
                                         xb[:, base + 58 * di + dj:
                                            base + 58 * di + dj + 464],
                                         start=(t == 0), stop=(t == 8))
                    src = pcv[:, 0:464].rearrange(
                        "p (r c) -> p r c", c=58)[:, :, 1:57]
                    dst = yq[ch][:, 448 * k:448 * (k + 1)].rearrange(
                        "p (r c) -> p r c", c=56)
                    nc.vector.tensor_scalar_add(dst, src, wb[ch][:, 27:28])
                # k/v: stride 2, 2 chunks of 14 rows (392 cols each)
                for cv, ykv in ((1, yk[ch]), (2, yv[ch])):
                    for r0 in (0, 14):
                        pcv = psAB.tile([128, 512], F32, tag="pcv",
                                        name="pcv", bufs=4)
                        for t, (di, dj) in enumerate(TAPS):
                            mv = x3[:, 1 + 2 * r0 + di:1 + 2 * r0 + di + 28:2,
                                    1 + dj:1 + dj + 56:2]
                            nc.tensor.matmul(
                                pcv[:, 0:392],
                                wd[ch][:, (9 * cv + t) * 128:
                                       (9 * cv + t + 1) * 128],
                                mv, start=(t == 0), stop=(t == 8))
                        nc.vector.tensor_scalar_add(
                            ykv[:, r0 * 28:r0 * 28 + 392], pcv[:, 0:392],
                            wb[ch][:, 27 + cv:28 + cv])

            # ---- Phase B: projections (bf16, double-buffered 1024 PSUM) ----
            for co in range(3):
                for g in range(4):
                    grp = LC[2 * g:2 * g + 2]
                    p = psAB.tile([128, 1024], F32, tag="psB", name="psB",
                                  bufs=2)
                    for k, (lo, ls) in enumerate(grp):
                        for ch in range(3):
                            nc.tensor.matmul(
                                p[0:128, k * 512:k * 512 + ls],
                                wq[ch][:, co * 128:(co + 1) * 128],
                                yq[ch][:, lo:lo + ls],
                                start=(ch == 0), stop=(ch == 2))
                    base = grp[0][0]
                    wid = grp[-1][0] + grp[-1][1] - base
                    nc.vector.tensor_copy(QT[co][:, base:base + wid],
                                          p[:, 0:wid])
                p = psAB.tile([128, 1024], F32, tag="psB", name="psB", bufs=2)
                for k, (to, ts) in enumerate(((0, 512), (512, 272))):
                    for ch in range(3):
                        nc.tensor.matmul(
                            p[:, k * 512:k * 512 + ts],
                            wk[ch][:, co * 128:(co + 1) * 128],
                            yk[ch][:, to:to + ts],
                            start=(ch == 0), stop=(ch == 2))
                nc.vector.tensor_copy(KT[co][:], p[:, 0:TKV])
            for g in range(4):
                tt = T_TILES[2 * g:2 * g + 2]
                p = psAB.tile([128, 1024], F32, tag="psB", name="psB", bufs=2)
                for k, (to, ts) in enumerate(tt):
                    for ch in range(3):
                        nc.tensor.matmul(
                            p[0:ts, k * 512:k * 512 + NH * 65],
                            yv[ch][:, to:to + ts], wvp[ch][:],
                            start=(ch == 0), stop=(ch == 2))
                for k, (to, ts) in enumerate(tt):
                    ti = 2 * g + k
                    nc.vector.tensor_copy(
                        Vh[ti][0:ts, :], p[0:ts, k * 512:k * 512 + NH * 65])
                    nc.vector.memset(Vh[ti][0:ts, 64:NH * 65:65], 1.0)

        # ---- Phase C: attention + pipelined normalize/out-projection ----
        with contextlib.ExitStack() as phC:
            cw = phC.enter_context(tc.tile_pool(name="cw" + sfx, bufs=2))
            psS = phC.enter_context(
                tc.tile_pool(name="psS" + sfx, bufs=1, space="PSUM"))

            def emit_tail(st):
                lo, ls, OTb, rcf = st
                rc2 = cw.tile([2, 1536], F32, tag="rc2", name="rc2")
                rcA = cw.tile([2, 1536], F32, tag="rcA", name="rcA")
                rc2r = cw.tile([2, 1536], BF16, tag="rc2r", name="rc2r")
                rcf3 = rcf[0:1, :].rearrange("p (g l) -> p g l", l=1024)
                rc2v = rc2[:].rearrange("p (g l) -> p g l", l=512)
                for q in range(2):
                    nc.sync.dma_start(rc2v[q:q + 1, :, 0:ls],
                                      rcf3[:, :, 512 * q:512 * q + ls])
                nc.vector.reciprocal_approx_fast(rcA[:], rc2[:])
                with nc.allow_low_precision(reason="bf16 softmax recip"):
                    nc.vector.tensor_copy(rc2r[:], rcA[:])
                osb = cw.tile([128, 2048], F32, tag="o", name="o")
                nlt = (ls + 127) // 128
                for c2 in range(3):
                    rb = psS.tile([128, 512], F32, tag="ps1", name="ps1")
                    nc.tensor.matmul(rb[:, 0:ls], ind2[:],
                                     rc2r[0:2, c2 * 512:c2 * 512 + ls],
                                     start=True, stop=True)
                    nc.vector.tensor_mul(OTb[:, c2 * 512:c2 * 512 + ls],
                                         OTb[:, c2 * 512:c2 * 512 + ls],
                                         rb[:, 0:ls])
                for k in range(nlt):
                    lsz = min(128, ls - k * 128)
                    win = psS.tile([128, 512], F32, tag="ps1", name="ps1")
                    for ch in range(3):
                        nc.tensor.matmul(
                            win[0:lsz, 0:C],
                            OTb[:, ch * 512 + k * 128:
                                ch * 512 + k * 128 + lsz],
                            wpj[ch][:], start=(ch == 0), stop=(ch == 2))
                    nc.vector.tensor_add(osb[0:lsz, k * 512:k * 512 + C],
                                         win[0:lsz, 0:C],
                                         bpjW[0:lsz, k * 512:k * 512 + C])
                ov = osb[:].rearrange("p (w c) -> p w c", c=512)[:, 0:nlt, 0:C]
                lsz = min(128, ls - (nlt - 1) * 128)
                if lsz < 128:
                    ov = ov[0:lsz]
                dst = d["out"][lo:lo + ls, :].rearrange(
                    "(w p) c -> p w c", p=min(128, ls))
                nc.sync.dma_start(dst, ov)

            prev = None
            for lo, ls in LC:
                OTb = cw.tile([128, 1536], BF16, tag="otb", name="otb")
                rcf = cw.tile([1, NH * 512], F32, tag="rcf", name="rcf")
                for h in range(NH):
                    c2, po = h // 2, 64 * (h % 2)
                    ets = []
                    for gi, tt in enumerate((T_TILES[0:4], T_TILES[4:7])):
                        wid = (len(tt) - 1) * 512 + ls
                        p = psS.tile([128, 2048 if gi == 0 else 1536], F32,
                                     tag=("psA" if gi == 0 else "psSB"),
                                     name="psS")
                        for k, (to, ts) in enumerate(tt):
                            nc.tensor.matmul(p[0:ts, k * 512:k * 512 + ls],
                                             KT[c2][po:po + 64, to:to + ts],
                                             QT[c2][po:po + 64, lo:lo + ls],
                                             start=True, stop=True)
                        et = cw.tile([128, 2048], BF16, tag="et", name="et",
                                     bufs=4)
                        nc.scalar.activation(et[:, 0:wid], p[:, 0:wid],
                                             AFT.Exp, scale=float(SCALE))
                        ets.append(et)
                    po2 = psS.tile([128, 512], F32, tag="ps1", name="ps1")
                    for ti, (to, ts) in enumerate(T_TILES):
                        nc.tensor.matmul(
                            po2[0:65, 0:ls], Vh[ti][0:ts, h * 65:(h + 1) * 65],
                            ets[ti // 4][0:ts, (ti % 4) * 512:(ti % 4) * 512 + ls],
                            start=(ti == 0), stop=(ti == 6))
                    nc.vector.tensor_copy(
                        OTb[po:po + 64, c2 * 512:c2 * 512 + ls], po2[0:64, 0:ls])
                    nc.vector.tensor_copy(rcf[0:1, h * 512:h * 512 + ls],
                                          po2[64:65, 0:ls])
                    if h == 2 and prev is not None:
                        emit_tail(prev)
                        prev = None
                prev = (lo, ls, OTb, rcf)
            emit_tail(prev)


def _build(reps=1):
    if reps in _CACHE:
        return _CACHE[reps]
    nc = bacc.Bacc("TRN2", target_bir_lowering=False, debug=False)
    d = {
        "xb": nc.dram_tensor("xb", [C, XB], BF16, kind="ExternalInput").ap(),
        "wb": nc.dram_tensor("wb", [3, 128, 30], F32, kind="ExternalInput").ap(),
        "wd": nc.dram_tensor("wd", [3, 128, 27 * 128], BF16,
                             kind="ExternalInput").ap(),
        "wq": nc.dram_tensor("wq", [C, C], BF16, kind="ExternalInput").ap(),
        "wk": nc.dram_tensor("wk", [C, C], BF16, kind="ExternalInput").ap(),
        "wvp": nc.dram_tensor("wvp", [C, NH * 65], BF16,
                              kind="ExternalInput").ap(),
        "wpj": nc.dram_tensor("wpj", [C, C], BF16, kind="ExternalInput").ap(),
        "ind2": nc.dram_tensor("ind2", [2, 128], BF16,
                               kind="ExternalInput").ap(),
        "bpjW": nc.dram_tensor("bpjW", [128, 2048], F32, kind="ExternalInput").ap(),
        "out": nc.dram_tensor("out", [T, C], F32, kind="ExternalOutput").ap(),
    }
    with tile.TileContext(nc) as tc:
        with contextlib.ExitStack() as ctx:
            _emit(nc, tc, ctx, d, reps)
    nc.compile()
    _CACHE[reps] = nc
    return nc


def _bpjw(bproj):
    w = np.zeros((128, 2048), np.float32)
    for k in range(4):
        w[:, k * 512:k * 512 + C] = bproj[None, :]
    return w


def _host_prep(x, conv_q, conv_k, conv_v, bn_q, bn_k, bn_v, Wq, Wk, Wv,
               Wproj, bproj):
    bf = ml_dtypes.bfloat16
    B = x.shape[0]
    x = np.asarray(x, np.float32)
    # 58x58 zero-padded bf16 image: data (r,c) at col 2 + (1+r)*58 + 1+c
    xb = np.zeros((B, C, XB), bf)
    xi = np.ascontiguousarray(x.transpose(0, 2, 1)).reshape(B, C, 56, 56)
    xb3 = xb[:, :, 2:2 + 3364].reshape(B, C, 58, 58)
    xb3[:, :, 1:57, 1:57] = xi.astype(bf)

    wb = np.zeros((3, 128, 30), np.float32)
    whs = []
    for cv, (w, bn) in enumerate(((conv_q, bn_q), (conv_k, bn_k),
                                  (conv_v, bn_v))):
        g, b, m, v = [np.asarray(bn[i], np.float64) for i in range(4)]
        a = g / np.sqrt(v + EPS)
        bias = (b - m * a).astype(np.float32)
        wh = (np.asarray(w, np.float64).reshape(C, 9) * a[:, None]).astype(
            np.float32)
        whs.append(wh)
        for ch in range(3):
            wb[ch, :, 9 * cv:9 * cv + 9] = wh[ch * 128:(ch + 1) * 128]
            wb[ch, :, 27 + cv] = bias[ch * 128:(ch + 1) * 128]

    # diag-packed conv weights for the PE:
    # wd[ch][p, (9*cv+t)*128 + q] = delta_pq * wh_cv[ch*128+p, t]
    wd = np.zeros((3, 128, 27 * 128), np.float32)
    idx = np.arange(128)
    for ch in range(3):
        for cv in range(3):
            for t in range(9):
                wd[ch, idx, (9 * cv + t) * 128 + idx] = \
                    whs[cv][ch * 128 + idx, t]

    wvp = np.zeros((C, NH * 65), np.float32)
    Wv = np.asarray(Wv, np.float32)
    for h in range(NH):
        wvp[:, h * 65:h * 65 + 64] = Wv[:, h * 64:(h + 1) * 64]

    ind2 = np.zeros((2, 128), np.float32)
    ind2[0, 0:64] = 1.0
    ind2[1, 64:128] = 1.0

    return {
        "xb": xb,
        "wb": wb,
        "wd": wd.astype(bf),
        "wq": np.asarray(Wq, np.float32).astype(bf),
        "wk": np.asarray(Wk, np.float32).astype(bf),
        "wvp": wvp.astype(bf),
        "wpj": np.asarray(Wproj, np.float32).astype(bf),
        "ind2": ind2.astype(bf),
        "bpjW": _bpjw(np.asarray(bproj, np.float32)),
    }


def kernel(x, h, w, conv_q, conv_k, conv_v, bn_q, bn_k, bn_v, Wq, Wk, Wv,
           Wproj, bproj, _reps=1, _nc=None):
    B = x.shape[0]
    nc = _nc if _nc is not None else _build(_reps)
    hp = _host_prep(x, conv_q, conv_k, conv_v, bn_q, bn_k, bn_v, Wq, Wk, Wv,
                    Wproj, bproj)
    shared = {k: v for k, v in hp.items() if k != "xb"}
    in_maps = [dict(shared, xb=hp["xb"][b]) for b in range(B)]
    res = run_bass_kernel_spmd(nc, in_maps, core_ids=list(range(B)))
    out = np.stack([res.results[b]["out"] for b in range(B)], axis=0)
    return out.astype(np.float32)
